# revision 1
# baseline (speedup 1.0000x reference)
"""Trainium2 Bass kernel: Gaussian-splat covariance from (scaling, rotation).

Math (per point n):
  s   = sigmoid(scaling)*(SMAX-SMIN) + SMIN                      # [3]
  q   = rotation / ||rotation||;  r,x,y,z = q
  R   = quaternion rotation matrix (3x3)
  L   = R @ diag(s);  C = L @ L^T;  out = upper-tri 6 of C

Implemented with unnormalized quaternion algebra:
  a,b,c,d = r^2,x^2,y^2,z^2 ; n2 = a+b+c+d
  Ru (row-major, = R*n2):
    [D0 E1 E2 / E3 D1 E4 / E5 E6 D2]
    D0=a+b-c-d  D1=a-b+c-d  D2=a-b-c+d
    E1=2xy-2rz E2=2xz+2ry E3=2xy+2rz E4=2yz-2rx E5=2xz-2ry E6=2yz+2rx
  K_j = s_j / n2 ;  L_ij = K_j * Ru_ij ;  C_ik = sum_j L_ij*L_kj

Layout: 8-way data parallel over points. Per core, tiles of 128x F points,
all per-point vectors interleaved along the free dim (strided views).
"""

import numpy as np

import concourse.bass as bass
import concourse.mybir as mybir
from concourse.tile import TileContext

F32 = mybir.dt.float32
ALU = mybir.AluOpType
ACTF = mybir.ActivationFunctionType

SCALE_MIN = 1e-4
SCALE_MAX = 10.0
A_SC = SCALE_MAX - SCALE_MIN
B_SC = SCALE_MIN

N_CORES = 8
N_TOTAL = 4_000_000

# Per-core tiling: P_CORE = 128 * F * T points.
F_PTS = 392
T_TILES = 10
P_CORE = 128 * F_PTS * T_TILES  # 501760; 8 cores cover 4,014,080 >= 4e6


def _v(tile_ap, k, start, count, step=1):
    """View of an interleaved tile [128, k*F]: per-point element sequence
    starting at `start`, `count` elements `step` apart -> [128, F, count]."""
    r = tile_ap.rearrange("p (f k) -> p f k", k=k)
    if count == 1:
        return r[:, :, start : start + 1]
    return r[:, :, start : start + (count - 1) * step + 1 : step]


def _bcast(tile_ap, k, pos, count):
    """Broadcast element `pos` of a k-interleaved tile across `count` lanes
    per point -> [128, F, count] with stride-0 inner."""
    r = tile_ap.rearrange("p (f k) -> p f k", k=k)
    one = r[:, :, pos : pos + 1]
    return one.broadcast_to((one.shape[0], one.shape[1], count))


def _split_sync_waits(nc, nop_max=1):
    """This container's walrus encodes at most 2 sync waits per instruction
    (and none on Drain). Move excess waits onto dedicated NoOps upstream."""
    n = 0
    for bb in nc.main_func.blocks:
        out = []
        for ins in bb.instructions:
            si = ins.sync_info
            waits = list(si.on_wait) if (si is not None and si.on_wait) else []
            is_drain = type(ins).__name__ == "InstDrain"
            limit = 0 if is_drain and len(waits) > 1 else 1
            if len(waits) > limit:
                keep = waits[-limit:] if limit else []
                extra = waits[:-limit] if limit else waits
                for i0 in range(0, len(extra), nop_max):
                    n += 1
                    nop = mybir.InstNoOp(name=f"waitsplit_{n}", ins=[], outs=[])
                    nop.engine = ins.engine
                    nop.sync_info = mybir.SyncInfo(
                        on_wait=extra[i0 : i0 + nop_max], on_update=[]
                    )
                    out.append(nop)
                ins.sync_info = mybir.SyncInfo(
                    on_wait=keep, on_update=list(si.on_update or [])
                )
            out.append(ins)
        bb.instructions[:] = out
    return n


def build_nc(F=F_PTS, T=T_TILES, pool_split=True, split_waits=True):
    """Build the per-core Bass program. Same program on all 8 cores."""
    nc = bass.Bass()
    P = 128
    npts = P * F * T

    rot_d = nc.declare_dram_parameter("rotation", [npts, 4], F32, isOutput=False)
    scal_d = nc.declare_dram_parameter("scaling", [npts, 3], F32, isOutput=False)
    out_d = nc.declare_dram_parameter("symm", [npts, 6], F32, isOutput=True)

    with TileContext(nc) as tc:
        with (
            tc.tile_pool(name="io", bufs=2) as io,
            tc.tile_pool(name="mid2", bufs=2) as mid2,
            tc.tile_pool(name="big1", bufs=1) as big1,
        ):
            for t in range(T):
                rows = slice(t * P * F, (t + 1) * P * F)

                ROT = io.tile([P, 4 * F], F32, tag="rot")
                SCAL = io.tile([P, 3 * F], F32, tag="scal")
                OUT = io.tile([P, 6 * F], F32, tag="out")
                nc.sync.dma_start(
                    ROT[:], rot_d[rows, :].rearrange("(p f) c -> p (f c)", p=P)
                )
                nc.sync.dma_start(
                    SCAL[:], scal_d[rows, :].rearrange("(p f) c -> p (f c)", p=P)
                )

                SQ = mid2.tile([P, 4 * F], F32, tag="sq")
                HAD = mid2.tile([P, 4 * F], F32, tag="had")
                N2 = mid2.tile([P, F], F32, tag="n2")
                INV2 = mid2.tile([P, F], F32, tag="inv2")
                PRD = mid2.tile([P, 6 * F], F32, tag="prd")
                SIG = mid2.tile([P, 3 * F], F32, tag="sig")
                K = mid2.tile([P, 3 * F], F32, tag="k")
                TD = mid2.tile([P, 3 * F], F32, tag="td")
                RU = big1.tile([P, 9 * F], F32, tag="ru")
                L = big1.tile([P, 9 * F], F32, tag="l")
                LSQ = big1.tile([P, 9 * F], F32, tag="lsq")

                # engine handles: ve = DVE-only ops, p1/p2 = splittable work
                ve = nc.vector
                pool = nc.gpsimd if pool_split else nc.vector

                # 1) squares of quaternion comps: SQ = [rr xx yy zz]
                nc.scalar.activation(SQ[:], ROT[:], ACTF.Square)

                # 2) Hadamard stage A -> HAD = [p pm q qm]
                #    p=rr+xx q=yy+zz pm=rr-xx qm=yy-zz
                pool.tensor_tensor(
                    _v(HAD[:], 4, 0, 2, 2), _v(SQ[:], 4, 0, 2, 2),
                    _v(SQ[:], 4, 1, 2, 2), ALU.add,
                )
                pool.tensor_tensor(
                    _v(HAD[:], 4, 1, 2, 2), _v(SQ[:], 4, 0, 2, 2),
                    _v(SQ[:], 4, 1, 2, 2), ALU.subtract,
                )
                # 3) stage B: n2 = p+q ; (D0,D2) = (p-q, pm-qm) ; D1 = pm+qm
                pool.tensor_tensor(
                    N2[:].unsqueeze(2), _v(HAD[:], 4, 0, 1), _v(HAD[:], 4, 2, 1),
                    ALU.add,
                )
                pool.tensor_tensor(
                    _v(RU[:], 9, 0, 2, 8), _v(HAD[:], 4, 0, 2, 1),
                    _v(HAD[:], 4, 2, 2, 1), ALU.subtract,
                )
                pool.tensor_tensor(
                    _v(RU[:], 9, 4, 1), _v(HAD[:], 4, 1, 1), _v(HAD[:], 4, 3, 1),
                    ALU.add,
                )

                # 4) INV2 = 1/n2
                ve.reciprocal(INV2[:], N2[:])

                # 5) doubled products PRD = [prx pry prz pxz pxy pyz]
                #    P1: (prx,pxy)=(2x*r, 2x*y)  P2: (pry,prz)=(2r*y, 2r*z)
                #    P3: (pxz,pyz)=(2z*x, 2z*y)
                ve.scalar_tensor_tensor(
                    _v(PRD[:], 6, 0, 2, 4), _bcast(ROT[:], 4, 1, 2), 2.0,
                    _v(ROT[:], 4, 0, 2, 2), ALU.mult, ALU.mult,
                )
                ve.scalar_tensor_tensor(
                    _v(PRD[:], 6, 1, 2, 1), _bcast(ROT[:], 4, 0, 2), 2.0,
                    _v(ROT[:], 4, 2, 2, 1), ALU.mult, ALU.mult,
                )
                ve.scalar_tensor_tensor(
                    _v(PRD[:], 6, 3, 2, 2), _bcast(ROT[:], 4, 3, 2), 2.0,
                    _v(ROT[:], 4, 1, 2, 1), ALU.mult, ALU.mult,
                )

                # 6) E terms into RU
                #    (E2,E3) = (pxz+pry, pxy+prz) -> RU(2,3)
                pool.tensor_tensor(
                    _v(RU[:], 9, 2, 2, 1), _v(PRD[:], 6, 3, 2, 1),
                    _v(PRD[:], 6, 1, 2, 1), ALU.add,
                )
                # E1 = pxy - prz -> RU(1)
                pool.tensor_tensor(
                    _v(RU[:], 9, 1, 1), _v(PRD[:], 6, 4, 1), _v(PRD[:], 6, 2, 1),
                    ALU.subtract,
                )
                # E4 = pyz - prx -> RU(5)
                pool.tensor_tensor(
                    _v(RU[:], 9, 5, 1), _v(PRD[:], 6, 5, 1), _v(PRD[:], 6, 0, 1),
                    ALU.subtract,
                )
                # E5 = pxz - pry -> RU(6)
                pool.tensor_tensor(
                    _v(RU[:], 9, 6, 1), _v(PRD[:], 6, 3, 1), _v(PRD[:], 6, 1, 1),
                    ALU.subtract,
                )
                # E6 = pyz + prx -> RU(7)
                pool.tensor_tensor(
                    _v(RU[:], 9, 7, 1), _v(PRD[:], 6, 5, 1), _v(PRD[:], 6, 0, 1),
                    ALU.add,
                )

                # 7) SIG = sigmoid(scaling) ; K = (SIG*A + B) * inv2
                nc.scalar.activation(SIG[:], SCAL[:], ACTF.Sigmoid)
                inv_rep3 = (
                    INV2[:].unsqueeze(2).broadcast_to((P, F, 3))
                )
                ve.tensor_scalar(K[:], SIG[:], A_SC, B_SC, ALU.mult, ALU.add)
                k3 = K[:].rearrange("p (f k) -> p f k", k=3)
                ve.tensor_tensor(k3, k3, inv_rep3, ALU.mult)

                # 8) L = RU * K(repeated over rows)
                ru4 = RU[:].rearrange("p (f i j) -> p f i j", i=3, j=3)
                k_rep = (
                    K[:].rearrange("p (f j) -> p f j", j=3)
                    .unsqueeze(2)
                    .broadcast_to((P, F, 3, 3))
                )
                l4 = L[:].rearrange("p (f i j) -> p f i j", i=3, j=3)
                ve.tensor_tensor(l4, ru4, k_rep, ALU.mult)

                # 9) LSQ = L^2
                nc.scalar.activation(LSQ[:], L[:], ACTF.Square)

                # 10) diagonal: Cii = LSQ[i0]+LSQ[i1]+LSQ[i2] -> OUT(0,3,5)
                lsq4 = LSQ[:].rearrange("p (f i j) -> p f i j", i=3, j=3)
                td3 = TD[:].rearrange("p (f i) -> p f i", i=3)
                ve.tensor_tensor(td3, lsq4[:, :, :, 0], lsq4[:, :, :, 1], ALU.add)
                ve.tensor_tensor(
                    _v(OUT[:], 6, 0, 2, 3), _v(TD[:], 3, 0, 2, 1),
                    _v(LSQ[:], 9, 2, 2, 3), ALU.add,
                )
                ve.tensor_tensor(
                    _v(OUT[:], 6, 5, 1), _v(TD[:], 3, 2, 1), _v(LSQ[:], 9, 8, 1),
                    ALU.add,
                )

                # 11) off-diagonals: C_ab = sum_j L[a,j]*L[b,j]
                #     PPall = [P01_0..2 P02_0..2 P12_0..2]; batched sums
                PPALL = mid2.tile([P, 9 * F], F32, tag="ppall")
                U3 = mid2.tile([P, 3 * F], F32, tag="u3")
                for pi, (ra, rb) in enumerate(((0, 1), (0, 2), (1, 2))):
                    ve.tensor_tensor(
                        _v(PPALL[:], 9, 3 * pi, 3, 1), l4[:, :, ra, :],
                        l4[:, :, rb, :], ALU.mult,
                    )
                ve.tensor_tensor(
                    U3[:].rearrange("p (f i) -> p f i", i=3),
                    _v(PPALL[:], 9, 0, 3, 3), _v(PPALL[:], 9, 1, 3, 3), ALU.add,
                )
                ve.tensor_tensor(
                    _v(OUT[:], 6, 1, 2, 1), _v(U3[:], 3, 0, 2, 1),
                    _v(PPALL[:], 9, 2, 2, 3), ALU.add,
                )
                ve.tensor_tensor(
                    _v(OUT[:], 6, 4, 1), _v(U3[:], 3, 2, 1), _v(PPALL[:], 9, 8, 1),
                    ALU.add,
                )

                # 12) store
                nc.sync.dma_start(
                    out_d[rows, :].rearrange("(p f) c -> p (f c)", p=P), OUT[:]
                )
    if split_waits:
        _split_sync_waits(nc)
    return nc


_NC_CACHE = {}


def _get_nc(F, T, pool_split=True):
    key = (F, T, pool_split)
    if key not in _NC_CACHE:
        _NC_CACHE[key] = build_nc(F, T, pool_split)
    return _NC_CACHE[key]


def kernel(scaling: np.ndarray, rotation: np.ndarray) -> np.ndarray:
    from concourse.bass_utils import run_bass_kernel_spmd

    scaling = np.ascontiguousarray(np.asarray(scaling, dtype=np.float32))
    rotation = np.ascontiguousarray(np.asarray(rotation, dtype=np.float32))
    n = scaling.shape[0]

    ntot = N_CORES * P_CORE
    scal_p = np.zeros((ntot, 3), dtype=np.float32)
    rot_p = np.zeros((ntot, 4), dtype=np.float32)
    rot_p[:, 0] = 1.0  # benign quaternion for padding
    scal_p[:n] = scaling
    rot_p[:n] = rotation

    nc = _get_nc(F_PTS, T_TILES)
    in_maps = [
        {
            "scaling": scal_p[i * P_CORE : (i + 1) * P_CORE],
            "rotation": rot_p[i * P_CORE : (i + 1) * P_CORE],
        }
        for i in range(N_CORES)
    ]
    res = run_bass_kernel_spmd(nc, in_maps, list(range(N_CORES)))
    out = np.concatenate([res.results[i]["symm"] for i in range(N_CORES)], axis=0)
    return out[:n]



# revision 2
# speedup vs baseline: 1.0337x; 1.0337x over previous
"""Trainium2 Bass kernel v2: Gaussian-splat covariance from (scaling, rotation).

Math (per point): s = sigmoid(sc)*(SMAX-SMIN)+SMIN; q normalized quaternion;
R = rot matrix; C = R diag(s^2) R^T; out = upper-tri-6 of C.

Rank-2 reformulation (saves computing the 3rd column of R):
  C = s2z*I + dxr*(a a^T)/n2^2 + dyr*(b b^T)/n2^2
  where a,b = first two UNNORMALIZED columns of R*n2, n2 = |q|^2,
  dxr = s2x - s2z, dyr = s2y - s2z.
With half-scaled squares (hc = c^2/2 via ACT Square(2^-1/2 * c)):
  A = a/2, B = b/2;  n2' = n2/2;  iv4 = (1/n2')^2 = 4/n2^2
  A0 = hr+hx-hy-hz = p-q,  A1 = xy+rz, A2 = xz-ry
  B0 = xy-rz, B1 = hr-hx+hy-hz = pm+qm, B2 = yz+rx
  C = s2z*I + (dxr*iv4)*(A A^T) + (dyr*iv4)*(B B^T)

Everything bf16 except n2'/iv2 (f32, for reciprocal_approx_fast) and the
final f32 outputs. Validated vs f64 reference in numpy: rel ~5.9e-3.

Layout: planar bf16 planes of F points per partition; inputs/outputs stay
interleaved f32 (contiguous DMA); de/interleave happens inside compute ops
via permuted access-pattern views.

Engine split: ACT does squares/sigmoid (contiguous, table set
'sigmoid_and_others' only - no table switches); DVE does the packed-bf16
2x-mode tensor_tensor work; Pool (gpsimd) does deinterleave copies, the
dxr/dyr chain and the f32 output stage.
"""

import numpy as np

import concourse.bass as bass
import concourse.mybir as mybir
from concourse.tile import TileContext

F32 = mybir.dt.float32
BF16 = mybir.dt.bfloat16
ALU = mybir.AluOpType
ACTF = mybir.ActivationFunctionType

SCALE_MIN = 1e-4
SCALE_MAX = 10.0
A_SC = SCALE_MAX - SCALE_MIN
B_SC = SCALE_MIN

N_CORES = 8
N_TOTAL = 4_000_000

# Per-core tiling: P_CORE = 128 * F * T points.
F_PTS = 784
T_TILES = 5
P_CORE = 128 * F_PTS * T_TILES  # 501760; 8 cores cover 4,014,080 >= 4e6


def _pl(tile_ap, k, i, n=1, step=1):
    """Planar tile [P, k*F] -> [P, n, F] view: planes i, i+step, ..."""
    r = tile_ap.rearrange("p (k f) -> p k f", k=k)
    if step == 1:
        return r[:, i : i + n]
    if step < 0:
        lo = i + (n - 1) * step
        return r[:, i : (lo - 1 if lo > 0 else None) : step]
    return r[:, i : i + (n - 1) * step + 1 : step]


def _plb(tile_ap, k, i, n):
    """Broadcast plane i of planar tile across n mid-lanes -> [P, n, F]."""
    r = tile_ap.rearrange("p (k f) -> p k f", k=k)
    one = r[:, i : i + 1]
    return one.broadcast_to((one.shape[0], n, one.shape[2]))


def _split_sync_waits(nc, nop_max=1):
    """This container's walrus encodes at most 2 sync waits per instruction
    (and none on Drain). Move excess waits onto dedicated NoOps upstream."""
    n = 0
    for bb in nc.main_func.blocks:
        out = []
        for ins in bb.instructions:
            si = ins.sync_info
            waits = list(si.on_wait) if (si is not None and si.on_wait) else []
            is_drain = type(ins).__name__ == "InstDrain"
            limit = 0 if is_drain and len(waits) > 1 else 1
            if len(waits) > limit:
                keep = waits[-limit:] if limit else []
                extra = waits[:-limit] if limit else waits
                for i0 in range(0, len(extra), nop_max):
                    n += 1
                    nop = mybir.InstNoOp(name=f"waitsplit_{n}", ins=[], outs=[])
                    nop.engine = ins.engine
                    nop.sync_info = mybir.SyncInfo(
                        on_wait=extra[i0 : i0 + nop_max], on_update=[]
                    )
                    out.append(nop)
                ins.sync_info = mybir.SyncInfo(
                    on_wait=keep, on_update=list(si.on_update or [])
                )
            out.append(ins)
        bb.instructions[:] = out
    return n


def build_nc(F=F_PTS, T=T_TILES, split_waits=True):
    """Build the per-core Bass program. Same program on all 8 cores."""
    nc = bass.Bass()
    P = 128
    npts = P * F * T

    # register the scale-activation bias constant (activation() lowers float
    # biases through the const-AP database, which only pre-registers 0/1)
    _bconst = nc.alloc_sbuf_tensor("const-f32-bsc", [P, 1], F32)
    nc.gpsimd.memset(_bconst.ap(), B_SC)
    nc.const_aps.aps[(F32, B_SC)] = _bconst.ap()
    nc.all_engine_barrier()

    rot_d = nc.declare_dram_parameter("rotation", [npts, 4], F32, isOutput=False)
    scal_d = nc.declare_dram_parameter("scaling", [npts, 3], F32, isOutput=False)
    out_d = nc.declare_dram_parameter("symm", [npts, 6], F32, isOutput=True)

    ve = nc.vector
    act = nc.scalar
    po = nc.gpsimd

    def emit_output_stage(C6, S2I, OUT, rows):
        """bf16 C6 planes + s2z -> interleaved f32 OUT, then DMA.
        Runs one tile behind compute (software pipelining) so the slow
        Pool/ACT tail overlaps the next tile's Vector work. All ops
        iterate plane-major so the C6 reads are unit-stride."""
        P = 128
        F = C6.shape[1] // 6
        outk = OUT[:].rearrange("p (f c) -> p c f", c=6)
        c6k = C6[:].rearrange("p (k f) -> p k f", k=6)
        s2zk = S2I[:].rearrange("p (f c) -> p c f", c=3)[:, 2:3]
        po.tensor_tensor(
            outk[:, 0:4:3], c6k[:, 0:4:3],
            s2zk.broadcast_to((P, 2, F)), ALU.add,
        )
        po.tensor_tensor(outk[:, 5:6], c6k[:, 5:6], s2zk, ALU.add)
        act.copy(outk[:, 1:3], c6k[:, 1:3])
        act.copy(outk[:, 4:5], c6k[:, 4:5])
        nc.sync.dma_start(
            out_d[rows, :].rearrange("(p f) c -> p (f c)", p=128), OUT[:]
        )

    with TileContext(nc) as tc:
        with (
            tc.tile_pool(name="io", bufs=2) as io,
            tc.tile_pool(name="acto", bufs=2) as acto,
            tc.tile_pool(name="s2ip", bufs=3) as s2ip,
            tc.tile_pool(name="c6p", bufs=3) as c6p,
            tc.tile_pool(name="work", bufs=1) as work,
        ):
            prev = None
            for t in range(T):
                rows = slice(t * P * F, (t + 1) * P * F)

                ROT = io.tile([P, 4 * F], F32, tag="rot")
                SCAL = io.tile([P, 3 * F], F32, tag="scal")
                OUT = io.tile([P, 6 * F], F32, tag="out")
                nc.sync.dma_start(
                    ROT[:], rot_d[rows, :].rearrange("(p f) c -> p (f c)", p=P)
                )
                nc.sync.dma_start(
                    SCAL[:], scal_d[rows, :].rearrange("(p f) c -> p (f c)", p=P)
                )

                # ACT outputs (double-buffered so ACT can run ahead)
                SQP = acto.tile([P, 4 * F], BF16, tag="sqp")  # hr hx hy hz
                SGI = acto.tile([P, 3 * F], BF16, tag="sgi")  # sigmoid, ilv
                S2I = s2ip.tile([P, 3 * F], BF16, tag="s2i")  # s^2, ilv
                # work tiles
                QP = work.tile([P, 4 * F], BF16, tag="qp")  # r x y z planes
                PQ = work.tile([P, 2 * F], BF16, tag="pq")  # p q
                PM = work.tile([P, 2 * F], BF16, tag="pm")  # pm qm
                N2 = work.tile([P, F], F32, tag="n2")
                LNN = work.tile([P, F], BF16, tag="lnn")
                IV4 = work.tile([P, F], BF16, tag="iv4")
                PRD = work.tile([P, 6 * F], BF16, tag="prd")  # xy xz ry rz rx yz
                ABT = work.tile([P, 6 * F], BF16, tag="abt")  # A0 A1 A2 B0 B1 B2
                D = work.tile([P, 2 * F], BF16, tag="d")  # dxr dyr
                TU = work.tile([P, 12 * F], BF16, tag="tu")
                C6 = c6p.tile([P, 6 * F], BF16, tag="c6")

                rot_perm = ROT[:].rearrange("p (f c) -> p c f", c=4)

                # --- DVE: deinterleave quaternion (f32 -> bf16 planes) ---
                ve.tensor_copy(_pl(QP[:], 4, 0, 4), rot_perm)

                # --- ACT: squares & sigmoid chain (one table set).
                # SQP reads contiguous QP planes - a strided read from ROT
                # costs 2.2x on ACT; squaring bf16-rounded values instead
                # of f32 is within noise.
                act.activation(
                    _pl(SQP[:], 4, 0, 4), _pl(QP[:], 4, 0, 4), ACTF.Square,
                    scale=2**-0.5,
                )
                act.activation(SGI[:], SCAL[:], ACTF.Sigmoid)
                act.activation(S2I[:], SGI[:], ACTF.Square, bias=B_SC, scale=A_SC)

                # --- DVE: butterflies ---
                # (p,q) = (hr,hy)+(hx,hz) ; (pm,qm) = (hr,hy)-(hx,hz)
                ve.tensor_tensor(
                    _pl(PQ[:], 2, 0, 2), _pl(SQP[:], 4, 0, 2, 2),
                    _pl(SQP[:], 4, 1, 2, 2), ALU.add,
                )
                ve.tensor_tensor(
                    _pl(PM[:], 2, 0, 2), _pl(SQP[:], 4, 0, 2, 2),
                    _pl(SQP[:], 4, 1, 2, 2), ALU.subtract,
                )
                # n2' = p+q (f32 out for the Ln input; stays on DVE - it
                # heads the critical n2->Ln->Exp->DD chain)
                ve.tensor_tensor(
                    N2[:].unsqueeze(1), _pl(PQ[:], 2, 0), _pl(PQ[:], 2, 1),
                    ALU.add,
                )
                # iv4 = (1/n2')^2 via ACT tables: exp(-2*ln(n2'))
                # (custom-DVE reciprocal ops don't compile on this walrus)
                act.activation(LNN[:], N2[:], ACTF.Ln)
                act.activation(IV4[:], LNN[:], ACTF.Exp, scale=-2.0)

                # A0 = p-q ; B1 = pm+qm
                ve.tensor_tensor(
                    _pl(ABT[:], 6, 0), _pl(PQ[:], 2, 0), _pl(PQ[:], 2, 1),
                    ALU.subtract,
                )
                ve.tensor_tensor(
                    _pl(ABT[:], 6, 4), _pl(PM[:], 2, 0), _pl(PM[:], 2, 1),
                    ALU.add,
                )

                # --- DVE: products (xy,xz) (ry,rz) (rx,yz) ---
                ve.tensor_tensor(
                    _pl(PRD[:], 6, 0, 2), _plb(QP[:], 4, 1, 2),
                    _pl(QP[:], 4, 2, 2), ALU.mult,
                )
                ve.tensor_tensor(
                    _pl(PRD[:], 6, 2, 2), _plb(QP[:], 4, 0, 2),
                    _pl(QP[:], 4, 2, 2), ALU.mult,
                )
                ve.tensor_tensor(
                    _pl(PRD[:], 6, 4, 2), _pl(QP[:], 4, 0, 2, 2),
                    _pl(QP[:], 4, 1, 2, 2), ALU.mult,
                )
                # (A1,B2) = (xy,yz)+(rz,rx) -> ABT planes (1,5)
                ve.tensor_tensor(
                    _pl(ABT[:], 6, 1, 2, 4), _pl(PRD[:], 6, 0, 2, 5),
                    _pl(PRD[:], 6, 3, 2, 1), ALU.add,
                )
                # (B0,A2) = (xy,xz)-(rz,ry) -> ABT planes (3,2)
                ve.tensor_tensor(
                    _pl(ABT[:], 6, 3, 2, -1), _pl(PRD[:], 6, 0, 2, 1),
                    _pl(PRD[:], 6, 3, 2, -1), ALU.subtract,
                )

                # --- Pool: dxr/dyr (reads interleaved S2I directly) ---
                s2i_cv = S2I[:].rearrange("p (f c) -> p c f", c=3)
                po.tensor_tensor(
                    _pl(D[:], 2, 0, 2), s2i_cv[:, 0:2],
                    s2i_cv[:, 2:3].broadcast_to((P, 2, F)), ALU.subtract,
                )
                # DD on DVE: keeps the PAB dependency on-engine (the Pool
                # version arrived late and stalled PAB)
                ve.tensor_tensor(
                    _pl(PM[:], 2, 0, 2), _pl(D[:], 2, 0, 2),
                    IV4[:].unsqueeze(1).broadcast_to((P, 2, F)), ALU.mult,
                )

                # --- DVE: PAB = ABT * DD (broadcast over 3 planes) ---
                abt_g = ABT[:].rearrange("p (g c f) -> p g c f", g=2, c=3)
                pab_g = PRD[:].rearrange("p (g c f) -> p g c f", g=2, c=3)
                dd_b = (
                    PM[:].rearrange("p (g f) -> p g f", g=2)
                    .unsqueeze(2)
                    .broadcast_to((P, 2, 3, F))
                )
                ve.tensor_tensor(pab_g, abt_g, dd_b, ALU.mult)

                # --- DVE: TU pairs t_ik = PAB_i * ABT_k ---
                tu_g = TU[:].rearrange("p (g k f) -> p g k f", g=2, k=6)
                ve.tensor_tensor(
                    tu_g[:, :, 0:3],
                    pab_g[:, :, 0:1].broadcast_to((P, 2, 3, F)),
                    abt_g, ALU.mult,
                )
                ve.tensor_tensor(
                    tu_g[:, :, 3:5],
                    pab_g[:, :, 1:2].broadcast_to((P, 2, 2, F)),
                    abt_g[:, :, 1:3], ALU.mult,
                )
                ve.tensor_tensor(
                    tu_g[:, :, 5:6], pab_g[:, :, 2:3], abt_g[:, :, 2:3],
                    ALU.mult,
                )

                # --- DVE: C6 = t + u  (c00 c01 c02 c11 c12 c22) ---
                ve.tensor_tensor(
                    C6[:], TU[:, 0 : 6 * F], TU[:, 6 * F : 12 * F], ALU.add
                )

                # --- output stage of the PREVIOUS tile (software pipeline) ---
                if prev is not None:
                    emit_output_stage(*prev)
                prev = (C6, S2I, OUT, rows)
            emit_output_stage(*prev)
    if split_waits:
        _split_sync_waits(nc)
    return nc


_NC_CACHE = {}


def _get_nc(F, T):
    key = (F, T)
    if key not in _NC_CACHE:
        _NC_CACHE[key] = build_nc(F, T)
    return _NC_CACHE[key]


def kernel(scaling: np.ndarray, rotation: np.ndarray) -> np.ndarray:
    from concourse.bass_utils import run_bass_kernel_spmd

    scaling = np.ascontiguousarray(np.asarray(scaling, dtype=np.float32))
    rotation = np.ascontiguousarray(np.asarray(rotation, dtype=np.float32))
    n = scaling.shape[0]

    ntot = N_CORES * P_CORE
    scal_p = np.zeros((ntot, 3), dtype=np.float32)
    rot_p = np.zeros((ntot, 4), dtype=np.float32)
    rot_p[:, 0] = 1.0  # benign quaternion for padding
    scal_p[:n] = scaling
    rot_p[:n] = rotation

    nc = _get_nc(F_PTS, T_TILES)
    in_maps = [
        {
            "scaling": scal_p[i * P_CORE : (i + 1) * P_CORE],
            "rotation": rot_p[i * P_CORE : (i + 1) * P_CORE],
        }
        for i in range(N_CORES)
    ]
    res = run_bass_kernel_spmd(nc, in_maps, list(range(N_CORES)))
    out = np.concatenate([res.results[i]["symm"] for i in range(N_CORES)], axis=0)
    return out[:n]


# revision 3
# speedup vs baseline: 1.0357x; 1.0019x over previous
"""Trainium2 Bass kernel v7: Gaussian-splat covariance from (scaling, rotation).

Math (per point): s = sigmoid(sc)*(SMAX-SMIN)+SMIN; q normalized quaternion;
R = rot matrix; C = R diag(s^2) R^T; out = upper-tri-6 of C.

Rank-2 reformulation (the 3rd column of R is never materialized):
  C = s2z*I + (dxr*iv4)*(A A^T) + (dyr*iv4)*(B B^T)
  A = a/2, B = b/2 (a,b = first two unnormalized columns of R*n2)
  n2' = n2/2 (from ACT Square with scale 2^-1/2); iv4 = n2'^-2 = 4/n2^2
  A0 = p-q, A1 = xy+rz, A2 = xz-ry; B0 = xy-rz, B1 = pm+qm, B2 = yz+rx
  iv4 via ACT tables: exp(-2*ln(n2')).

All bf16 except n2' (f32 Ln input) and the f32 outputs. Validated vs f64
reference: rel ~5.9e-3 (tolerance 2e-2).

Layout: bf16 planes of f points per partition; HBM I/O stays interleaved
(contiguous DMA); de/interleave happens inside compute-op access patterns.
First/last tiles are split 4x smaller to shorten pipeline fill/drain.

Engine split: ACT = quaternion deinterleave cast + squares + sigmoid +
ln/exp + offdiag output casts; DVE = packed-bf16 2x tensor_tensor chain;
Pool = dxr/dyr + diag output adds. Output stage runs one tile behind
compute (software pipeline).
"""

import numpy as np

import concourse.bass as bass
import concourse.mybir as mybir
from concourse.tile import TileContext

F32 = mybir.dt.float32
BF16 = mybir.dt.bfloat16
ALU = mybir.AluOpType
ACTF = mybir.ActivationFunctionType

SCALE_MIN = 1e-4
SCALE_MAX = 10.0
A_SC = SCALE_MAX - SCALE_MIN
B_SC = SCALE_MIN

N_CORES = 8
N_TOTAL = 4_000_000

F_PTS = 784
T_TILES = 5
P_CORE = 128 * F_PTS * T_TILES  # 501760; 8 cores cover 4,014,080 >= 4e6


def _pl(tile_ap, k, f, i, n=1, step=1):
    """Planar tile view [P, k*f] -> [P, n, f]: planes i, i+step, ..."""
    r = tile_ap[:, : k * f].rearrange("p (k f) -> p k f", k=k)
    if step == 1:
        return r[:, i : i + n]
    if step < 0:
        lo = i + (n - 1) * step
        return r[:, i : (lo - 1 if lo > 0 else None) : step]
    return r[:, i : i + (n - 1) * step + 1 : step]


def _plb(tile_ap, k, f, i, n):
    """Broadcast plane i across n mid-lanes -> [P, n, f]."""
    r = tile_ap[:, : k * f].rearrange("p (k f) -> p k f", k=k)
    one = r[:, i : i + 1]
    return one.broadcast_to((one.shape[0], n, one.shape[2]))


def _split_sync_waits(nc, nop_max=1):
    """This container's walrus encodes at most 2 sync waits per instruction
    (and none on Drain). Move excess waits onto dedicated NoOps upstream."""
    n = 0
    for bb in nc.main_func.blocks:
        out = []
        for ins in bb.instructions:
            si = ins.sync_info
            waits = list(si.on_wait) if (si is not None and si.on_wait) else []
            is_drain = type(ins).__name__ == "InstDrain"
            limit = 0 if is_drain and len(waits) > 1 else 1
            if len(waits) > limit:
                keep = waits[-limit:] if limit else []
                extra = waits[:-limit] if limit else waits
                for i0 in range(0, len(extra), nop_max):
                    n += 1
                    nop = mybir.InstNoOp(name=f"waitsplit_{n}", ins=[], outs=[])
                    nop.engine = ins.engine
                    nop.sync_info = mybir.SyncInfo(
                        on_wait=extra[i0 : i0 + nop_max], on_update=[]
                    )
                    out.append(nop)
                ins.sync_info = mybir.SyncInfo(
                    on_wait=keep, on_update=list(si.on_update or [])
                )
            out.append(ins)
        bb.instructions[:] = out
    return n


def build_nc(F=F_PTS, T=T_TILES, split_waits=True, split_edge=4):
    """Build the per-core Bass program. Same program on all 8 cores."""
    nc = bass.Bass()
    P = 128
    npts = P * F * T

    _bconst = nc.alloc_sbuf_tensor("const-f32-bsc", [P, 1], F32)
    nc.gpsimd.memset(_bconst.ap(), B_SC)
    nc.const_aps.aps[(F32, B_SC)] = _bconst.ap()
    nc.all_engine_barrier()

    rot_d = nc.declare_dram_parameter("rotation", [npts, 4], F32, isOutput=False)
    scal_d = nc.declare_dram_parameter("scaling", [npts, 3], F32, isOutput=False)
    out_d = nc.declare_dram_parameter("symm", [npts, 6], F32, isOutput=True)

    ve = nc.vector
    act = nc.scalar
    po = nc.gpsimd

    # segment list: (row_start, f); first/last tile split to shorten
    # pipeline fill and drain
    segs = []
    for t in range(T):
        base = t * P * F
        if t in (0, T - 1) and split_edge > 1:
            q = F // split_edge
            segs += [(base + i * P * q, q) for i in range(split_edge)]
        else:
            segs.append((base, F))

    def emit_output_stage(C6, S2I, OUT, rows, f):
        """bf16 C6 planes + s2z -> interleaved f32 OUT, then DMA. Runs one
        segment behind compute. Plane-major iteration = unit-stride reads."""
        outk = OUT[:, : 6 * f].rearrange("p (f c) -> p c f", c=6)
        c6k = C6[:, : 6 * f].rearrange("p (k f) -> p k f", k=6)
        s2zk = S2I[:, : 3 * f].rearrange("p (f c) -> p c f", c=3)[:, 2:3]
        po.tensor_tensor(
            outk[:, 0:4:3], c6k[:, 0:4:3],
            s2zk.broadcast_to((P, 2, f)), ALU.add,
        )
        po.tensor_tensor(outk[:, 5:6], c6k[:, 5:6], s2zk, ALU.add)
        act.copy(outk[:, 1:3], c6k[:, 1:3])
        act.copy(outk[:, 4:5], c6k[:, 4:5])
        nc.sync.dma_start(
            out_d[rows, :].rearrange("(p f) c -> p (f c)", p=P), OUT[:, : 6 * f]
        )

    with TileContext(nc) as tc:
        with (
            tc.tile_pool(name="io", bufs=2) as io,
            tc.tile_pool(name="acto", bufs=2) as acto,
            tc.tile_pool(name="s2ip", bufs=3) as s2ip,
            tc.tile_pool(name="c6p", bufs=3) as c6p,
            tc.tile_pool(name="work", bufs=1) as work,
        ):
            prev = None
            for row0, f in segs:
                rows = slice(row0, row0 + P * f)

                ROT = io.tile([P, 4 * f], F32, tag="rot")
                SCAL = io.tile([P, 3 * f], F32, tag="scal")
                OUT = io.tile([P, 6 * f], F32, tag="out")
                nc.sync.dma_start(
                    ROT[:], rot_d[rows, :].rearrange("(p f) c -> p (f c)", p=P)
                )
                nc.sync.dma_start(
                    SCAL[:], scal_d[rows, :].rearrange("(p f) c -> p (f c)", p=P)
                )

                SQP = acto.tile([P, 4 * f], BF16, tag="sqp")  # hr hx hy hz
                SGI = acto.tile([P, 3 * f], BF16, tag="sgi")
                QP = acto.tile([P, 4 * f], BF16, tag="qp")  # r x y z planes
                S2I = s2ip.tile([P, 3 * f], BF16, tag="s2i")
                PQ = work.tile([P, 2 * f], BF16, tag="pq")  # p q
                PM = work.tile([P, 2 * f], BF16, tag="pm")  # pm qm -> later DD
                N2 = work.tile([P, f], F32, tag="n2")
                LNN = work.tile([P, f], BF16, tag="lnn")
                IV4 = work.tile([P, f], BF16, tag="iv4")
                PRD = work.tile([P, 6 * f], BF16, tag="prd")  # xy xz ry rz rx yz
                ABT = work.tile([P, 6 * f], BF16, tag="abt")  # A0 A1 A2 B0 B1 B2
                D = work.tile([P, 2 * f], BF16, tag="d")  # dxr dyr
                TU = work.tile([P, 12 * f], BF16, tag="tu")
                C6 = c6p.tile([P, 6 * f], BF16, tag="c6")

                rot_perm = ROT[:, : 4 * f].rearrange("p (f c) -> p c f", c=4)

                # --- deinterleave quaternion: split ACT (r,x) / DVE (y,z)
                # to balance engine load ---
                act.copy(_pl(QP, 4, f, 0, 2), rot_perm[:, 0:2])
                ve.tensor_copy(_pl(QP, 4, f, 2, 2), rot_perm[:, 2:4])
                act.activation(
                    _pl(SQP, 4, f, 0, 4), _pl(QP, 4, f, 0, 4), ACTF.Square,
                    scale=2**-0.5,
                )
                act.activation(SGI[:], SCAL[:], ACTF.Sigmoid)
                act.activation(S2I[:], SGI[:], ACTF.Square, bias=B_SC, scale=A_SC)

                # --- DVE: butterflies ---
                ve.tensor_tensor(
                    _pl(PQ, 2, f, 0, 2), _pl(SQP, 4, f, 0, 2, 2),
                    _pl(SQP, 4, f, 1, 2, 2), ALU.add,
                )
                ve.tensor_tensor(
                    _pl(PM, 2, f, 0, 2), _pl(SQP, 4, f, 0, 2, 2),
                    _pl(SQP, 4, f, 1, 2, 2), ALU.subtract,
                )
                ve.tensor_tensor(
                    N2[:].unsqueeze(1), _pl(PQ, 2, f, 0), _pl(PQ, 2, f, 1),
                    ALU.add,
                )
                act.activation(LNN[:], N2[:], ACTF.Ln)
                act.activation(IV4[:], LNN[:], ACTF.Exp, scale=-2.0)

                ve.tensor_tensor(
                    _pl(ABT, 6, f, 0), _pl(PQ, 2, f, 0), _pl(PQ, 2, f, 1),
                    ALU.subtract,
                )
                ve.tensor_tensor(
                    _pl(ABT, 6, f, 4), _pl(PM, 2, f, 0), _pl(PM, 2, f, 1),
                    ALU.add,
                )

                # --- DVE: products (xy,xz,ry,rz) fused + (rx,yz) ---
                prd4 = PRD[:, : 4 * f].rearrange("p (a b f) -> p a b f", a=2, b=2)
                xr = _pl(QP, 4, f, 1, 2, -1).unsqueeze(2).broadcast_to((P, 2, 2, f))
                yz2 = _pl(QP, 4, f, 2, 2).unsqueeze(1).broadcast_to((P, 2, 2, f))
                ve.tensor_tensor(prd4, xr, yz2, ALU.mult)
                ve.tensor_tensor(
                    _pl(PRD, 6, f, 4, 2), _pl(QP, 4, f, 0, 2, 2),
                    _pl(QP, 4, f, 1, 2, 2), ALU.mult,
                )
                # (A1,B2) = (xy,yz)+(rz,rx) -> ABT planes (1,5)
                ve.tensor_tensor(
                    _pl(ABT, 6, f, 1, 2, 4), _pl(PRD, 6, f, 0, 2, 5),
                    _pl(PRD, 6, f, 3, 2, 1), ALU.add,
                )
                # (B0,A2) = (xy,xz)-(rz,ry) -> ABT planes (3,2)
                ve.tensor_tensor(
                    _pl(ABT, 6, f, 3, 2, -1), _pl(PRD, 6, f, 0, 2, 1),
                    _pl(PRD, 6, f, 3, 2, -1), ALU.subtract,
                )

                # --- Pool: dxr/dyr ---
                s2i_cv = S2I[:, : 3 * f].rearrange("p (f c) -> p c f", c=3)
                po.tensor_tensor(
                    _pl(D, 2, f, 0, 2), s2i_cv[:, 0:2],
                    s2i_cv[:, 2:3].broadcast_to((P, 2, f)), ALU.subtract,
                )
                # DD -> overwrites PM (dead after B1); stays on DVE
                ve.tensor_tensor(
                    _pl(PM, 2, f, 0, 2), _pl(D, 2, f, 0, 2),
                    IV4[:].unsqueeze(1).broadcast_to((P, 2, f)), ALU.mult,
                )

                # --- DVE: PAB = ABT * DD (into PRD, dead after combines) ---
                abt_g = ABT[:, : 6 * f].rearrange("p (g c f) -> p g c f", g=2, c=3)
                pab_g = PRD[:, : 6 * f].rearrange("p (g c f) -> p g c f", g=2, c=3)
                dd_b = (
                    PM[:, : 2 * f].rearrange("p (g f) -> p g f", g=2)
                    .unsqueeze(2)
                    .broadcast_to((P, 2, 3, f))
                )
                ve.tensor_tensor(pab_g, abt_g, dd_b, ALU.mult)

                # --- DVE: TU pairs t_ik = PAB_i * ABT_k ---
                tu_g = TU[:, : 12 * f].rearrange("p (g k f) -> p g k f", g=2, k=6)
                ve.tensor_tensor(
                    tu_g[:, :, 0:3],
                    pab_g[:, :, 0:1].broadcast_to((P, 2, 3, f)),
                    abt_g, ALU.mult,
                )
                ve.tensor_tensor(
                    tu_g[:, :, 3:5],
                    pab_g[:, :, 1:2].broadcast_to((P, 2, 2, f)),
                    abt_g[:, :, 1:3], ALU.mult,
                )
                ve.tensor_tensor(
                    tu_g[:, :, 5:6], pab_g[:, :, 2:3], abt_g[:, :, 2:3],
                    ALU.mult,
                )

                # --- DVE: C6 = t + u ---
                ve.tensor_tensor(
                    C6[:, : 6 * f], TU[:, 0 : 6 * f], TU[:, 6 * f : 12 * f],
                    ALU.add,
                )

                if prev is not None:
                    emit_output_stage(*prev)
                prev = (C6, S2I, OUT, rows, f)
            emit_output_stage(*prev)
    if split_waits:
        _split_sync_waits(nc)
    return nc


_NC_CACHE = {}


def _get_nc(F, T):
    key = (F, T)
    if key not in _NC_CACHE:
        _NC_CACHE[key] = build_nc(F, T)
    return _NC_CACHE[key]


P = 128


def kernel(scaling: np.ndarray, rotation: np.ndarray) -> np.ndarray:
    from concourse.bass_utils import run_bass_kernel_spmd

    scaling = np.ascontiguousarray(np.asarray(scaling, dtype=np.float32))
    rotation = np.ascontiguousarray(np.asarray(rotation, dtype=np.float32))
    n = scaling.shape[0]

    ntot = N_CORES * P_CORE
    scal_p = np.zeros((ntot, 3), dtype=np.float32)
    rot_p = np.zeros((ntot, 4), dtype=np.float32)
    rot_p[:, 0] = 1.0  # benign quaternion for padding
    scal_p[:n] = scaling
    rot_p[:n] = rotation

    nc = _get_nc(F_PTS, T_TILES)
    in_maps = [
        {
            "scaling": scal_p[i * P_CORE : (i + 1) * P_CORE],
            "rotation": rot_p[i * P_CORE : (i + 1) * P_CORE],
        }
        for i in range(N_CORES)
    ]
    res = run_bass_kernel_spmd(nc, in_maps, list(range(N_CORES)))
    out = np.concatenate([res.results[i]["symm"] for i in range(N_CORES)], axis=0)
    return out[:n]


# revision 4
# speedup vs baseline: 1.0521x; 1.0158x over previous
"""Trainium2 Bass kernel v7: Gaussian-splat covariance from (scaling, rotation).

Math (per point): s = sigmoid(sc)*(SMAX-SMIN)+SMIN; q normalized quaternion;
R = rot matrix; C = R diag(s^2) R^T; out = upper-tri-6 of C.

Rank-2 reformulation (the 3rd column of R is never materialized):
  C = s2z*I + (dxr*iv4)*(A A^T) + (dyr*iv4)*(B B^T)
  A = a/2, B = b/2 (a,b = first two unnormalized columns of R*n2)
  n2' = n2/2 (from ACT Square with scale 2^-1/2); iv4 = n2'^-2 = 4/n2^2
  A0 = p-q, A1 = xy+rz, A2 = xz-ry; B0 = xy-rz, B1 = pm+qm, B2 = yz+rx
  iv4 via ACT tables: exp(-2*ln(n2')).

All bf16 except n2' (f32 Ln input) and the f32 outputs. Validated vs f64
reference: rel ~5.9e-3 (tolerance 2e-2).

Layout: bf16 planes of f points per partition; HBM I/O stays interleaved
(contiguous DMA); de/interleave happens inside compute-op access patterns.
First/last tiles are split 4x smaller to shorten pipeline fill/drain.

Engine split: ACT = quaternion deinterleave cast + squares + sigmoid +
ln/exp + offdiag output casts; DVE = packed-bf16 2x tensor_tensor chain;
Pool = dxr/dyr + diag output adds. Output stage runs one tile behind
compute (software pipeline).
"""

import numpy as np

import concourse.bass as bass
import concourse.mybir as mybir
from concourse.tile import TileContext

F32 = mybir.dt.float32
BF16 = mybir.dt.bfloat16
ALU = mybir.AluOpType
ACTF = mybir.ActivationFunctionType

SCALE_MIN = 1e-4
SCALE_MAX = 10.0
A_SC = SCALE_MAX - SCALE_MIN
B_SC = SCALE_MIN

N_CORES = 8
N_TOTAL = 4_000_000

F_PTS = 784
T_TILES = 5
P_CORE = 128 * F_PTS * T_TILES  # 501760; 8 cores cover 4,014,080 >= 4e6


def _pl(tile_ap, k, f, i, n=1, step=1):
    """Planar tile view [P, k*f] -> [P, n, f]: planes i, i+step, ..."""
    r = tile_ap[:, : k * f].rearrange("p (k f) -> p k f", k=k)
    if step == 1:
        return r[:, i : i + n]
    if step < 0:
        lo = i + (n - 1) * step
        return r[:, i : (lo - 1 if lo > 0 else None) : step]
    return r[:, i : i + (n - 1) * step + 1 : step]


def _plb(tile_ap, k, f, i, n):
    """Broadcast plane i across n mid-lanes -> [P, n, f]."""
    r = tile_ap[:, : k * f].rearrange("p (k f) -> p k f", k=k)
    one = r[:, i : i + 1]
    return one.broadcast_to((one.shape[0], n, one.shape[2]))


def _split_sync_waits(nc, nop_max=1):
    """This container's walrus encodes at most 2 sync waits per instruction
    (and none on Drain). Move excess waits onto dedicated NoOps upstream."""
    n = 0
    for bb in nc.main_func.blocks:
        out = []
        for ins in bb.instructions:
            si = ins.sync_info
            waits = list(si.on_wait) if (si is not None and si.on_wait) else []
            is_drain = type(ins).__name__ == "InstDrain"
            limit = 0 if is_drain and len(waits) > 1 else 1
            if len(waits) > limit:
                keep = waits[-limit:] if limit else []
                extra = waits[:-limit] if limit else waits
                for i0 in range(0, len(extra), nop_max):
                    n += 1
                    nop = mybir.InstNoOp(name=f"waitsplit_{n}", ins=[], outs=[])
                    nop.engine = ins.engine
                    nop.sync_info = mybir.SyncInfo(
                        on_wait=extra[i0 : i0 + nop_max], on_update=[]
                    )
                    out.append(nop)
                ins.sync_info = mybir.SyncInfo(
                    on_wait=keep, on_update=list(si.on_update or [])
                )
            out.append(ins)
        bb.instructions[:] = out
    return n


def build_nc(F=F_PTS, T=T_TILES, split_waits=True, split_edge=4):
    """Build the per-core Bass program. Same program on all 8 cores."""
    nc = bass.Bass()
    P = 128
    npts = P * F * T

    _bconst = nc.alloc_sbuf_tensor("const-f32-bsc", [P, 1], F32)
    nc.gpsimd.memset(_bconst.ap(), B_SC)
    nc.const_aps.aps[(F32, B_SC)] = _bconst.ap()
    nc.all_engine_barrier()

    rot_d = nc.declare_dram_parameter("rotation", [npts, 4], F32, isOutput=False)
    scal_d = nc.declare_dram_parameter("scaling", [npts, 3], F32, isOutput=False)
    out_d = nc.declare_dram_parameter("symm", [npts, 6], F32, isOutput=True)

    ve = nc.vector
    act = nc.scalar
    po = nc.gpsimd

    # segment list: (row_start, f); first/last tile split to shorten
    # pipeline fill and drain
    segs = []
    for t in range(T):
        base = t * P * F
        if t in (0, T - 1) and split_edge > 1:
            q = F // split_edge
            segs += [(base + i * P * q, q) for i in range(split_edge)]
        else:
            segs.append((base, F))

    def emit_output_stage(C6, S2I, OUT, rows, f):
        """bf16 C6 planes + s2z -> interleaved f32 OUT, then DMA. Runs one
        segment behind compute. Plane-major iteration = unit-stride reads."""
        outk = OUT[:, : 6 * f].rearrange("p (f c) -> p c f", c=6)
        c6k = C6[:, : 6 * f].rearrange("p (k f) -> p k f", k=6)
        s2zk = S2I[:, : 3 * f].rearrange("p (f c) -> p c f", c=3)[:, 2:3]
        po.tensor_tensor(
            outk[:, 0:4:3], c6k[:, 0:4:3],
            s2zk.broadcast_to((P, 2, f)), ALU.add,
        )
        po.tensor_tensor(outk[:, 5:6], c6k[:, 5:6], s2zk, ALU.add)
        act.copy(outk[:, 1:3], c6k[:, 1:3])
        act.copy(outk[:, 4:5], c6k[:, 4:5])
        nc.sync.dma_start(
            out_d[rows, :].rearrange("(p f) c -> p (f c)", p=P), OUT[:, : 6 * f]
        )

    with TileContext(nc) as tc:
        with (
            tc.tile_pool(name="io", bufs=2) as io,
            tc.tile_pool(name="acto", bufs=2) as acto,
            tc.tile_pool(name="s2ip", bufs=4) as s2ip,
            tc.tile_pool(name="c6p", bufs=3) as c6p,
            tc.tile_pool(name="work", bufs=1) as work,
        ):
            prev = None
            for row0, f in segs:
                rows = slice(row0, row0 + P * f)

                ROT = io.tile([P, 4 * f], F32, tag="rot")
                SCAL = io.tile([P, 3 * f], F32, tag="scal")
                OUT = io.tile([P, 6 * f], F32, tag="out")
                nc.sync.dma_start(
                    ROT[:], rot_d[rows, :].rearrange("(p f) c -> p (f c)", p=P)
                )
                nc.sync.dma_start(
                    SCAL[:], scal_d[rows, :].rearrange("(p f) c -> p (f c)", p=P)
                )

                SQP = acto.tile([P, 4 * f], BF16, tag="sqp")  # hr hx hy hz
                SGI = acto.tile([P, 3 * f], BF16, tag="sgi")
                QP = acto.tile([P, 4 * f], BF16, tag="qp")  # r x y z planes
                S2I = s2ip.tile([P, 3 * f], BF16, tag="s2i")
                PQ = work.tile([P, 2 * f], BF16, tag="pq")  # p q
                PM = work.tile([P, 2 * f], BF16, tag="pm")  # pm qm -> later DD
                N2 = work.tile([P, f], mybir.dt.float16, tag="n2")
                LNN = work.tile([P, f], BF16, tag="lnn")
                IV4 = work.tile([P, f], BF16, tag="iv4")
                PRD = work.tile([P, 6 * f], BF16, tag="prd")  # xy xz ry rz rx yz
                ABT = work.tile([P, 6 * f], BF16, tag="abt")  # A0 A1 A2 B0 B1 B2
                TU = work.tile([P, 12 * f], BF16, tag="tu")
                C6 = c6p.tile([P, 6 * f], BF16, tag="c6")

                rot_perm = ROT[:, : 4 * f].rearrange("p (f c) -> p c f", c=4)

                # --- deinterleave quaternion: split ACT (r,x) / DVE (y,z)
                # to balance engine load ---
                act.copy(_pl(QP, 4, f, 0, 2), rot_perm[:, 0:2])
                ve.tensor_copy(_pl(QP, 4, f, 2, 2), rot_perm[:, 2:4])
                act.activation(
                    _pl(SQP, 4, f, 0, 4), _pl(QP, 4, f, 0, 4), ACTF.Square,
                    scale=2**-0.5,
                )
                act.activation(SGI[:], SCAL[:], ACTF.Sigmoid)
                act.activation(S2I[:], SGI[:], ACTF.Square, bias=B_SC, scale=A_SC)

                # --- DVE: butterflies ---
                ve.tensor_tensor(
                    _pl(PQ, 2, f, 0, 2), _pl(SQP, 4, f, 0, 2, 2),
                    _pl(SQP, 4, f, 1, 2, 2), ALU.add,
                )
                ve.tensor_tensor(
                    _pl(PM, 2, f, 0, 2), _pl(SQP, 4, f, 0, 2, 2),
                    _pl(SQP, 4, f, 1, 2, 2), ALU.subtract,
                )
                ve.tensor_tensor(
                    N2[:].unsqueeze(1), _pl(PQ, 2, f, 0), _pl(PQ, 2, f, 1),
                    ALU.add,
                )
                act.activation(LNN[:], N2[:], ACTF.Ln)
                act.activation(IV4[:], LNN[:], ACTF.Exp, scale=-2.0)

                ve.tensor_tensor(
                    _pl(ABT, 6, f, 0), _pl(PQ, 2, f, 0), _pl(PQ, 2, f, 1),
                    ALU.subtract,
                )
                ve.tensor_tensor(
                    _pl(ABT, 6, f, 4), _pl(PM, 2, f, 0), _pl(PM, 2, f, 1),
                    ALU.add,
                )

                # --- DVE: products (xy,xz,ry,rz) fused + (rx,yz) ---
                prd4 = PRD[:, : 4 * f].rearrange("p (a b f) -> p a b f", a=2, b=2)
                xr = _pl(QP, 4, f, 1, 2, -1).unsqueeze(2).broadcast_to((P, 2, 2, f))
                yz2 = _pl(QP, 4, f, 2, 2).unsqueeze(1).broadcast_to((P, 2, 2, f))
                ve.tensor_tensor(prd4, xr, yz2, ALU.mult)
                ve.tensor_tensor(
                    _pl(PRD, 6, f, 4, 2), _pl(QP, 4, f, 0, 2, 2),
                    _pl(QP, 4, f, 1, 2, 2), ALU.mult,
                )
                # (A1,B2) = (xy,yz)+(rz,rx) -> ABT planes (1,5)
                ve.tensor_tensor(
                    _pl(ABT, 6, f, 1, 2, 4), _pl(PRD, 6, f, 0, 2, 5),
                    _pl(PRD, 6, f, 3, 2, 1), ALU.add,
                )
                # (B0,A2) = (xy,xz)-(rz,ry) -> ABT planes (3,2)
                ve.tensor_tensor(
                    _pl(ABT, 6, f, 3, 2, -1), _pl(PRD, 6, f, 0, 2, 1),
                    _pl(PRD, 6, f, 3, 2, -1), ALU.subtract,
                )

                # --- Pool: dxr/dyr ---
                s2i_cv = S2I[:, : 3 * f].rearrange("p (f c) -> p c f", c=3)
                po.tensor_tensor(
                    _pl(PQ, 2, f, 0, 2), s2i_cv[:, 0:2],
                    s2i_cv[:, 2:3].broadcast_to((P, 2, f)), ALU.subtract,
                )
                # DD -> overwrites PM (dead after B1); stays on DVE
                ve.tensor_tensor(
                    _pl(PM, 2, f, 0, 2), _pl(PQ, 2, f, 0, 2),
                    IV4[:].unsqueeze(1).broadcast_to((P, 2, f)), ALU.mult,
                )

                # --- DVE: PAB = ABT * DD (into PRD, dead after combines) ---
                abt_g = ABT[:, : 6 * f].rearrange("p (g c f) -> p g c f", g=2, c=3)
                pab_g = PRD[:, : 6 * f].rearrange("p (g c f) -> p g c f", g=2, c=3)
                dd_b = (
                    PM[:, : 2 * f].rearrange("p (g f) -> p g f", g=2)
                    .unsqueeze(2)
                    .broadcast_to((P, 2, 3, f))
                )
                ve.tensor_tensor(pab_g, abt_g, dd_b, ALU.mult)

                # --- DVE: TU pairs t_ik = PAB_i * ABT_k ---
                tu_g = TU[:, : 12 * f].rearrange("p (g k f) -> p g k f", g=2, k=6)
                ve.tensor_tensor(
                    tu_g[:, :, 0:3],
                    pab_g[:, :, 0:1].broadcast_to((P, 2, 3, f)),
                    abt_g, ALU.mult,
                )
                ve.tensor_tensor(
                    tu_g[:, :, 3:5],
                    pab_g[:, :, 1:2].broadcast_to((P, 2, 2, f)),
                    abt_g[:, :, 1:3], ALU.mult,
                )
                ve.tensor_tensor(
                    tu_g[:, :, 5:6], pab_g[:, :, 2:3], abt_g[:, :, 2:3],
                    ALU.mult,
                )

                # --- DVE: C6 = t + u ---
                ve.tensor_tensor(
                    C6[:, : 6 * f], TU[:, 0 : 6 * f], TU[:, 6 * f : 12 * f],
                    ALU.add,
                )

                if prev is not None:
                    emit_output_stage(*prev)
                prev = (C6, S2I, OUT, rows, f)
            emit_output_stage(*prev)
    if split_waits:
        _split_sync_waits(nc)
    return nc


_NC_CACHE = {}


def _get_nc(F, T):
    key = (F, T)
    if key not in _NC_CACHE:
        _NC_CACHE[key] = build_nc(F, T)
    return _NC_CACHE[key]


P = 128


def kernel(scaling: np.ndarray, rotation: np.ndarray) -> np.ndarray:
    from concourse.bass_utils import run_bass_kernel_spmd

    scaling = np.ascontiguousarray(np.asarray(scaling, dtype=np.float32))
    rotation = np.ascontiguousarray(np.asarray(rotation, dtype=np.float32))
    n = scaling.shape[0]

    ntot = N_CORES * P_CORE
    scal_p = np.zeros((ntot, 3), dtype=np.float32)
    rot_p = np.zeros((ntot, 4), dtype=np.float32)
    rot_p[:, 0] = 1.0  # benign quaternion for padding
    scal_p[:n] = scaling
    rot_p[:n] = rotation

    nc = _get_nc(F_PTS, T_TILES)
    in_maps = [
        {
            "scaling": scal_p[i * P_CORE : (i + 1) * P_CORE],
            "rotation": rot_p[i * P_CORE : (i + 1) * P_CORE],
        }
        for i in range(N_CORES)
    ]
    res = run_bass_kernel_spmd(nc, in_maps, list(range(N_CORES)))
    out = np.concatenate([res.results[i]["symm"] for i in range(N_CORES)], axis=0)
    return out[:n]
